# revision 22
# baseline (speedup 1.0000x reference)
"""DisRNN sampling kernel for 8x Trainium2 NeuronCores (Bass/Tile).

Strategy (pure data parallel over batch B=16384 -> 2048 per core):
  The dominant cost is streaming noise_update [B, 66, 64] (~277MB fp32).
  All scaling factors (sqrt(var_u), mult_u) are folded into the per-latent
  MLP weights on the host, so the noise tensor feeds the tensor engine
  directly with no elementwise pass:
    h1[b,z,:] = relu( sum_d noise[b,d,z]*W1n[z,d,:]      (grouped matmul)
                    + sum_d x[b,d]  *W1x[z,d,:] + b1 )   (dense matmul)
  z-latents are packed pairwise (z, z+32) into K=128 block-diagonal
  matmuls. During sharding the host lays the noise slice out
  partition-major and chunk-blocked ([128 = 2d+(z>=32)] partitions, 32KB
  contiguous runs) so the device DMA runs at full HBM bandwidth and the
  matmul moving operand is contiguous. MLP2/MLP3 are 8-wide
  block-diagonal matmuls; kld_u / kld_g reduce to tiny matmuls on
  x^2 / new_latents^2. Noise-path matmuls run bf16, the rest float32r;
  all accumulation is fp32 in PSUM.
"""

import sys

if "/opt/trn_rl_repo" not in sys.path:
    sys.path.insert(0, "/opt/trn_rl_repo")

import numpy as np
import ml_dtypes

B, Z, NOBS = 16384, 64, 2
D = Z + NOBS
H1, H2 = 16, 16
HC1, HC2 = 32, 32
NCORES = 8
BL = B // NCORES          # 2048 per core
CHS = [512, 512, 512, 384, 128]   # per-chunk batch sizes (sum = BL)
CHB = [0]
for _c in CHS:
    CHB.append(CHB[-1] + _c)
assert CHB[-1] == BL
NCH = len(CHS)
CH = 512                  # max chunk (PSUM free dim)

BF16 = ml_dtypes.bfloat16
PAIR_TILEPOS = False      # 4-way concurrent pair matmuls (32x32 col strips)

# packed parameter blob column layouts (cols; all blobs have 128 partitions)
#   pba (bf16): wpair [32*PW];  pbb (bf16): wobs [8*128] | w2 [8*128] | wx [8*128]
#   pr (f32r): w3 [8*128] | kw [64] | kg [1] | wc1 [32] | wc2 [32] | wc3 [2]
#   pf (f32):  b1p [8] | b2p [8] | b3u,b3w,kbu,mg,sdg,kgb [6] | bc1,bc2,bc3 [3]
PW = 128
PBA_COLS = 32 * PW
PBB_COLS = 8 * 128 + 8 * 128 + 8 * 128
PR_COLS = 8 * 128 + 64 + 1 + 32 + 32 + 2
PF_COLS = 8 + 8 + 6 + 3

_cache = {}


def _z_of(g, zl):
    return (4 * g + zl // 2) + 32 * (zl % 2)


def _build():
    import concourse.bass as bass
    import concourse.mybir as mybir
    import concourse.tile as tile
    from concourse import bacc

    f32 = mybir.dt.float32
    f32r = mybir.dt.float32r
    bf16 = mybir.dt.bfloat16
    AF = mybir.ActivationFunctionType
    OP = mybir.AluOpType

    nc = bacc.Bacc("TRN2", target_bir_lowering=False, debug=False)

    nzP = nc.dram_tensor("nzP", [128, 32 * BL], bf16, kind="ExternalInput")
    obsP = nc.dram_tensor("obsP", [128, BL], bf16, kind="ExternalInput")
    xT = nc.dram_tensor("xT", [D, BL], f32, kind="ExternalInput")
    xTb = nc.dram_tensor("xTb", [D, BL], bf16, kind="ExternalInput")
    ngT = nc.dram_tensor("ngT", [Z, BL], f32, kind="ExternalInput")
    pba = nc.dram_tensor("pba", [128, PBA_COLS], bf16, kind="ExternalInput")
    pbb = nc.dram_tensor("pbb", [128, PBB_COLS], bf16, kind="ExternalInput")
    pr = nc.dram_tensor("pr", [128, PR_COLS], f32r, kind="ExternalInput")
    pf = nc.dram_tensor("pf", [128, PF_COLS], f32, kind="ExternalInput")

    yT = nc.dram_tensor("yT", [2, BL], f32, kind="ExternalOutput")
    ztT = nc.dram_tensor("ztT", [Z, BL], f32, kind="ExternalOutput")
    kldg = nc.dram_tensor("kldg", [1, BL], f32, kind="ExternalOutput")
    klduT = nc.dram_tensor("klduT", [Z, BL], f32, kind="ExternalOutput")

    def rf(ap):
        return ap.bitcast(f32)

    with tile.TileContext(nc) as tc:
        with (
            tc.tile_pool(name="singles", bufs=1) as singles,
            tc.tile_pool(name="nz", bufs=2) as nzp,
            tc.tile_pool(name="h1", bufs=3) as h1p,
            tc.tile_pool(name="h2", bufs=3) as h2p,
            tc.tile_pool(name="wio", bufs=2) as wio,
            tc.tile_pool(name="wtmp", bufs=2) as wtmp,
            tc.tile_pool(name="psm1", bufs=3, space="PSUM") as psm1,
            tc.tile_pool(name="psm2", bufs=2, space="PSUM") as psm2,
            tc.tile_pool(name="psout", bufs=2, space="PSUM") as psout,
            tc.tile_pool(name="pssm", bufs=1, space="PSUM") as pssm,
        ):
            # ---- parameter blobs on the scalar HWDGE queue, by first use ----
            pba_s = singles.tile([128, PBA_COLS], bf16, tag="pba")
            nc.scalar.dma_start(out=pba_s, in_=pba[:, :])
            pf_s = singles.tile([128, PF_COLS], f32, tag="pf")
            nc.scalar.dma_start(out=pf_s, in_=pf[:, :])
            pbb_s = singles.tile([128, PBB_COLS], bf16, tag="pbb")
            nc.scalar.dma_start(out=pbb_s, in_=pbb[:, :])
            xTb_s = singles.tile([D, BL], bf16, tag="xTb")
            nc.scalar.dma_start(out=xTb_s, in_=xTb[:, :])
            pr_s = singles.tile([128, PR_COLS], f32r, tag="pr")
            nc.scalar.dma_start(out=pr_s, in_=pr[:, :])
            xT_s = singles.tile([D, BL], f32, tag="xT")
            nc.scalar.dma_start(out=xT_s, in_=xT[:, :])
            ngT_s = singles.tile([Z, BL], f32, tag="ngT")
            nc.scalar.dma_start(out=ngT_s, in_=ngT[:, :])

            wpair_v = [pba_s[:, q * PW:(q + 1) * PW] for q in range(32)]
            o = 0
            wobs_v = [pbb_s[:, o + g * 128: o + (g + 1) * 128] for g in range(8)]
            o += 8 * 128
            w2_v = [pbb_s[:, o + g * 128: o + (g + 1) * 128] for g in range(8)]
            o += 8 * 128
            wx_v = [pbb_s[0:D, o + g * 128: o + (g + 1) * 128] for g in range(8)]

            o = 0
            w3_v = [pr_s[:, o + g * 128: o + (g + 1) * 128] for g in range(8)]
            o += 8 * 128
            kw_s = pr_s[0:D, o:o + Z]; o += Z
            kg_s = pr_s[0:Z, o:o + 1]; o += 1
            wc1_s = pr_s[0:Z, o:o + HC1]; o += HC1
            wc2_s = pr_s[0:HC1, o:o + HC2]; o += HC2
            wc3_s = pr_s[0:HC2, o:o + 2]

            o = 0
            b1_v = [pf_s[:, o + g: o + g + 1] for g in range(8)]; o += 8
            b2_v = [pf_s[:, o + g: o + g + 1] for g in range(8)]; o += 8
            b3u_s = pf_s[0:Z, o:o + 1]; o += 1
            b3w_s = pf_s[0:Z, o:o + 1]; o += 1
            kbu_s = pf_s[0:Z, o:o + 1]; o += 1
            mg_s = pf_s[0:Z, o:o + 1]; o += 1
            sdg_s = pf_s[0:Z, o:o + 1]; o += 1
            kgb_s = pf_s[0:1, o:o + 1]; o += 1
            bc1_s = pf_s[0:HC1, o:o + 1]; o += 1
            bc2_s = pf_s[0:HC2, o:o + 1]; o += 1
            bc3_s = pf_s[0:2, o:o + 1]

            xsq_s = singles.tile([D, BL], f32r, tag="xsq")
            nc.vector.tensor_mul(xsq_s, xT_s, xT_s)

            for c in range(NCH):
                boff = CHB[c]
                CHc = CHS[c]
                # ---- noise DMA: 32KB contiguous runs per partition ----
                nsub = 8 if c == 0 else 4
                zw = 32 // nsub
                co = 32 * boff
                subs = []
                for kk in range(nsub):
                    nzk = nzp.tile([128, zw, CHc], bf16,
                                   tag=f"nz{'A' if c == 0 else 'B'}{kk}",
                                   bufs=1 if c == 0 else None)
                    nc.sync.dma_start(
                        out=nzk,
                        in_=nzP[:, co + kk * zw * CHc:co + (kk + 1) * zw * CHc])
                    subs.append(nzk)
                nzs = [(subs[q // zw], q % zw) for q in range(32)]
                noT = wio.tile([128, CHc], bf16, tag="noT")
                obs_ap = bass.AP(
                    tensor=obsP[:, :].tensor, offset=boff,
                    ap=[[BL, 128], [1, CHc]],
                )
                nc.sync.dma_start(out=noT, in_=obs_ap)

                latT_c = xT_s[0:Z, boff:boff + CHc]

                out_ps = psout.tile([128, CHc], f32, tag="outps")
                h1s, h2s = {}, {}

                def emit_m1(g):
                    m1 = psm1.tile([128, CHc], f32, tag="m1")
                    for j in range(4):
                        q = 4 * g + j
                        nc.tensor.matmul(
                            m1, wpair_v[q], nzs[q][0][:, nzs[q][1], :],
                            start=(j == 0), stop=False, skip_group_check=True,
                        )
                    nc.tensor.matmul(
                        m1, wx_v[g], xTb_s[:, boff:boff + CHc],
                        start=False, stop=False, skip_group_check=True,
                    )
                    nc.tensor.matmul(
                        m1, wobs_v[g], noT,
                        start=False, stop=True, skip_group_check=True,
                    )
                    h1 = h1p.tile([128, CHc], bf16, tag="h1")
                    if g % 2 == 0:
                        nc.scalar.activation(h1, m1, AF.Relu,
                                             bias=b1_v[g], scale=1.0)
                    else:
                        nc.vector.tensor_scalar(
                            out=h1, in0=m1, scalar1=b1_v[g],
                            scalar2=0.0, op0=OP.add, op1=OP.max)
                    h1s[g] = h1

                def emit_m2(g):
                    m2 = psm2.tile([128, CHc], f32, tag="m2")
                    nc.tensor.matmul(m2, w2_v[g], h1s[g], start=True, stop=True)
                    h2 = h2p.tile([128, CHc], f32r, tag="h2")
                    if g % 2 == 1:
                        nc.scalar.activation(h2, m2, AF.Relu,
                                             bias=b2_v[g], scale=1.0)
                    else:
                        nc.vector.tensor_scalar(
                            out=h2, in0=m2, scalar1=b2_v[g],
                            scalar2=0.0, op0=OP.add, op1=OP.max)
                    h2s[g] = h2

                def emit_m3(g):
                    nc.tensor.matmul(out_ps, w3_v[g], h2s[g],
                                     start=(g == 0), stop=(g == 7),
                                     skip_group_check=True)

                for g in range(8):
                    emit_m1(g)
                    emit_m2(g)
                    emit_m3(g)

                # ---- gated update; out_ps partitions: [0:64]=u, [64:128]=w
                w_t = wtmp.tile([Z, CHc], f32, tag="wt")
                nc.scalar.activation(w_t, out_ps[Z:128, :], AF.Sigmoid,
                                     bias=b3w_s, scale=1.0)
                t1 = wtmp.tile([Z, CHc], f32, tag="t1")
                nc.vector.scalar_tensor_tensor(
                    out=t1, in0=out_ps[0:Z, :], scalar=b3u_s, in1=latT_c,
                    op0=OP.add, op1=OP.subtract)
                t2 = wtmp.tile([Z, CHc], f32, tag="t2")
                nc.vector.tensor_mul(t2, t1, w_t)
                nl = wtmp.tile([Z, CHc], f32, tag="nl")
                nc.vector.tensor_add(nl, t2, latT_c)

                # ---- z_tilde = sd_g*noise_g + mult_g*new_lat
                t3 = wtmp.tile([Z, CHc], f32, tag="t3")
                nc.vector.tensor_scalar(
                    out=t3, in0=ngT_s[:, boff:boff + CHc], scalar1=sdg_s,
                    scalar2=None, op0=OP.mult)
                zt = wio.tile([Z, CHc], f32, tag="zt")
                nc.vector.scalar_tensor_tensor(
                    out=zt, in0=nl, scalar=mg_s, in1=t3,
                    op0=OP.mult, op1=OP.add)
                nc.scalar.dma_start(out=ztT[:, boff:boff + CHc], in_=zt)
                ztr = wtmp.tile([Z, CHc], f32r, tag="ztr")
                nc.vector.tensor_copy(ztr, zt)

                # ---- kld_u = 0.5*sum_d x^2 mult^2 - 0.5*C[z]
                ku_ps = pssm.tile([Z, CHc], f32, tag="sm")
                nc.tensor.matmul(ku_ps, kw_s, xsq_s[:, boff:boff + CHc],
                                 start=True, stop=True)
                ku = wio.tile([Z, CHc], f32, tag="ku")
                nc.vector.tensor_scalar(out=ku, in0=ku_ps, scalar1=kbu_s,
                                        scalar2=None, op0=OP.add)
                nc.scalar.dma_start(out=klduT[:, boff:boff + CHc], in_=ku)


                # ---- kld_g = 0.5*sum_z mult_g^2 nl^2 - 0.5*Cg
                nlsq = wtmp.tile([Z, CHc], f32r, tag="nlsq")
                nc.vector.tensor_mul(nlsq, nl, nl)
                kg_ps = pssm.tile([1, CHc], f32, tag="sm")
                nc.tensor.matmul(kg_ps, kg_s, nlsq, start=True, stop=True)
                kgo = wio.tile([1, CHc], f32, tag="kgo")
                nc.vector.tensor_scalar(out=kgo, in0=kg_ps, scalar1=kgb_s,
                                        scalar2=None, op0=OP.add)
                nc.scalar.dma_start(out=kldg[0:1, boff:boff + CHc], in_=kgo)

                # ---- choice MLP on z_tilde
                c1_ps = pssm.tile([HC1, CHc], f32, tag="sm")
                nc.tensor.matmul(c1_ps, wc1_s, ztr, start=True, stop=True)
                c1s = wtmp.tile([HC1, CHc], f32r, tag="c1s")
                nc.scalar.activation(c1s, c1_ps, AF.Relu, bias=bc1_s, scale=1.0)
                c2_ps = pssm.tile([HC2, CHc], f32, tag="sm")
                nc.tensor.matmul(c2_ps, wc2_s, c1s, start=True, stop=True)
                c2s = wtmp.tile([HC2, CHc], f32r, tag="c2s")
                nc.scalar.activation(c2s, c2_ps, AF.Relu, bias=bc2_s, scale=1.0)
                y_ps = pssm.tile([2, CHc], f32, tag="sm")
                nc.tensor.matmul(y_ps, wc3_s, c2s, start=True, stop=True)
                ys = wio.tile([2, CHc], f32, tag="ys")
                nc.scalar.activation(ys, y_ps, AF.Identity, bias=bc3_s, scale=1.0)
                nc.scalar.dma_start(out=yT[:, boff:boff + CHc], in_=ys)

    nc.compile()
    return nc


def _prep_params(inp):
    """Host-side folding of the tiny (<1MB) parameters into packed blobs."""
    f = np.float32
    logvar_u = np.asarray(inp["logvar_u"], f)       # [D, Z]
    mult_u = np.asarray(inp["mult_u"], f)           # [D, Z]
    W1 = np.asarray(inp["W1"], f)                   # [Z, D, H1]
    b1 = np.asarray(inp["b1"], f)
    W2 = np.asarray(inp["W2"], f)
    b2 = np.asarray(inp["b2"], f)
    W3 = np.asarray(inp["W3"], f)
    b3 = np.asarray(inp["b3"], f)
    logvar_g = np.asarray(inp["logvar_g"], f)
    mult_g = np.asarray(inp["mult_g"], f)

    var_u = np.exp(logvar_u)
    sd_u = np.sqrt(var_u)
    W1n = sd_u.T[:, :, None] * W1                   # [Z, D, H1]
    W1x = mult_u.T[:, :, None] * W1

    wpair = np.zeros((128, 32, PW), f)
    for q in range(32):
        co = 32 * (q % 4)
        for d in range(Z):
            wpair[2 * d, q, co:co + 16] = W1n[q, d, :]
            wpair[2 * d + 1, q, co + 16:co + 32] = W1n[q + 32, d, :]

    wobs = np.zeros((128, 8, 128), f)
    wx = np.zeros((128, 8, 128), f)
    b1p = np.zeros((128, 8), f)
    w2p = np.zeros((128, 8, 128), f)
    b2p = np.zeros((128, 8), f)
    w3p = np.zeros((128, 8, 128), f)
    for g in range(8):
        for zl in range(8):
            z = _z_of(g, zl)
            s = 16 * zl
            for dd in range(2):
                wobs[dd * Z + z, g, s:s + 16] = W1n[z, Z + dd, :]
            wx[:D, g, s:s + 16] = W1x[z]
            b1p[s:s + 16, g] = b1[z]
            w2p[s:s + 16, g, s:s + 16] = W2[z]
            b2p[s:s + 16, g] = b2[z]
            for o in range(2):
                w3p[s:s + 16, g, z + Z * o] = W3[z, :, o]

    C = np.sum(1.0 + logvar_u - var_u, axis=0)
    var_g = np.exp(logvar_g)
    Cg = np.sum(1.0 + logvar_g - var_g)

    # ---- pack blobs ----
    pba = wpair.reshape(128, 32 * PW).astype(BF16)
    pbb = np.concatenate([
        wobs.reshape(128, 8 * 128),
        w2p.reshape(128, 8 * 128),
        wx.reshape(128, 8 * 128),
    ], axis=1).astype(BF16)

    pr = np.zeros((128, PR_COLS), f)
    o = 0
    pr[:, o:o + 8 * 128] = w3p.reshape(128, 8 * 128); o += 8 * 128
    pr[:D, o:o + Z] = 0.5 * mult_u * mult_u; o += Z
    pr[:Z, o] = 0.5 * mult_g * mult_g; o += 1
    pr[:Z, o:o + HC1] = np.asarray(inp["Wc1"], f); o += HC1
    pr[:HC1, o:o + HC2] = np.asarray(inp["Wc2"], f); o += HC2
    pr[:HC2, o:o + 2] = np.asarray(inp["Wc3"], f)

    pf = np.zeros((128, PF_COLS), f)
    o = 0
    pf[:, o:o + 8] = b1p; o += 8
    pf[:, o:o + 8] = b2p; o += 8
    pf[:Z, o] = b3[:, 0]; o += 1
    pf[:Z, o] = b3[:, 1]; o += 1
    pf[:Z, o] = -0.5 * C; o += 1
    pf[:Z, o] = mult_g; o += 1
    pf[:Z, o] = np.sqrt(var_g); o += 1
    pf[0, o] = -0.5 * Cg; o += 1
    pf[:HC1, o] = np.asarray(inp["bc1"], f); o += 1
    pf[:HC2, o] = np.asarray(inp["bc2"], f); o += 1
    pf[:2, o] = np.asarray(inp["bc3"], f)

    return {"pba": pba, "pbb": pbb, "pr": pr, "pf": pf}


def _layout_noise(noise_core):
    """[BL, 66, 64] -> nzP [128, 32*BL] bf16 (p = 2d + (z>=32),
    chunk-blocked, b contiguous) and obsP [128, BL] bf16."""
    blocks = []
    for c in range(NCH):
        blk = noise_core[CHB[c]:CHB[c + 1], :Z, :].reshape(CHS[c], Z, 2, 32)
        blocks.append(blk.transpose(1, 2, 3, 0).reshape(128, 32 * CHS[c]))
    nzP = np.ascontiguousarray(np.concatenate(blocks, axis=1).astype(BF16))
    ob = noise_core[:, Z:, :]                        # [BL, 2, 64]
    obsP = np.ascontiguousarray(
        ob.transpose(1, 2, 0).reshape(128, BL).astype(BF16))
    return nzP, obsP


def kernel(**inputs):
    from concourse.bass_utils import run_bass_kernel_spmd

    if "nc" not in _cache:
        _cache["nc"] = _build()
    nc = _cache["nc"]

    f = np.float32
    latents = np.asarray(inputs["latents"], f)
    obs = np.asarray(inputs["obs"], f)
    t_0 = int(np.asarray(inputs["t_0"]))
    if t_0:
        lat = np.broadcast_to(np.asarray(inputs["z_0"], f), latents.shape)
    else:
        lat = latents
    x = np.concatenate([lat, obs], axis=-1)          # [B, D]
    xT = np.ascontiguousarray(x.T)
    ngT = np.ascontiguousarray(np.asarray(inputs["noise_global"], f).T)
    noise = np.asarray(inputs["noise_update"], f)

    params = _prep_params(inputs)

    in_maps = []
    for c in range(NCORES):
        s = slice(c * BL, (c + 1) * BL)
        m = dict(params)
        m["nzP"], m["obsP"] = _layout_noise(noise[s])
        m["xT"] = np.ascontiguousarray(xT[:, s])
        m["xTb"] = m["xT"].astype(BF16)
        m["ngT"] = np.ascontiguousarray(ngT[:, s])
        in_maps.append(m)

    res = run_bass_kernel_spmd(nc, in_maps, core_ids=list(range(NCORES)))
    _cache["last_results"] = res
    _cache["last_in_maps"] = in_maps

    y = np.concatenate([r["yT"] for r in res.results], axis=1).T
    z_tilde = np.concatenate([r["ztT"] for r in res.results], axis=1).T
    kld_g = np.concatenate([r["kldg"][0] for r in res.results], axis=0)
    kld_u = np.concatenate([r["klduT"] for r in res.results], axis=1).T
    return (np.ascontiguousarray(y), np.ascontiguousarray(z_tilde),
            kld_g, np.ascontiguousarray(kld_u))


# revision 23
# speedup vs baseline: 1.0176x; 1.0176x over previous
"""DisRNN sampling kernel for 8x Trainium2 NeuronCores (Bass/Tile).

Strategy (pure data parallel over batch B=16384 -> 2048 per core):
  The dominant cost is streaming noise_update [B, 66, 64] (~277MB fp32).
  All scaling factors (sqrt(var_u), mult_u) are folded into the per-latent
  MLP weights on the host, so the noise tensor feeds the tensor engine
  directly with no elementwise pass:
    h1[b,z,:] = relu( sum_d noise[b,d,z]*W1n[z,d,:]      (grouped matmul)
                    + sum_d x[b,d]  *W1x[z,d,:] + b1 )   (dense matmul)
  z-latents are packed pairwise (z, z+32) into K=128 block-diagonal
  matmuls. During sharding the host lays the noise slice out
  partition-major and chunk-blocked ([128 = 2d+(z>=32)] partitions, 32KB
  contiguous runs) so the device DMA runs at full HBM bandwidth and the
  matmul moving operand is contiguous. MLP2/MLP3 are 8-wide
  block-diagonal matmuls; kld_u / kld_g reduce to tiny matmuls on
  x^2 / new_latents^2. Noise-path matmuls run bf16, the rest float32r;
  all accumulation is fp32 in PSUM.
"""

import sys

if "/opt/trn_rl_repo" not in sys.path:
    sys.path.insert(0, "/opt/trn_rl_repo")

import numpy as np
import ml_dtypes

B, Z, NOBS = 16384, 64, 2
D = Z + NOBS
H1, H2 = 16, 16
HC1, HC2 = 32, 32
NCORES = 8
BL = B // NCORES          # 2048 per core
CHS = [512, 512, 512, 512]   # per-chunk batch sizes (sum = BL)
CHB = [0]
for _c in CHS:
    CHB.append(CHB[-1] + _c)
assert CHB[-1] == BL
NCH = len(CHS)
CH = 512                  # max chunk (PSUM free dim)

BF16 = ml_dtypes.bfloat16

# packed parameter blob column layouts (cols; all blobs have 128 partitions)
#   pba (bf16): wpair [32*PW];  pbb (bf16): wobs [8*128] | w2 [8*128] | wx [8*128]
#   pr (f32r): w3 [8*128] | kw [64] | kg [1] | wc1 [32] | wc2 [32] | wc3 [2]
#   pf (f32):  b1p [8] | b2p [8] | b3u,b3w,kbu,mg,sdg,kgb [6] | bc1,bc2,bc3 [3]
PW = 128
PBA_COLS = 32 * PW
PBB_COLS = 8 * 128 + 8 * 128 + 8 * 128
PR_COLS = 8 * 128 + 64 + 1 + 32 + 32 + 2
PF_COLS = 8 + 8 + 6 + 3

_cache = {}


def _z_of(g, zl):
    return (4 * g + zl // 2) + 32 * (zl % 2)


def _build():
    import concourse.bass as bass
    import concourse.mybir as mybir
    import concourse.tile as tile
    from concourse import bacc

    f32 = mybir.dt.float32
    f32r = mybir.dt.float32r
    bf16 = mybir.dt.bfloat16
    AF = mybir.ActivationFunctionType
    OP = mybir.AluOpType

    nc = bacc.Bacc("TRN2", target_bir_lowering=False, debug=False)

    nzP = nc.dram_tensor("nzP", [128, 32 * BL], bf16, kind="ExternalInput")
    obsP = nc.dram_tensor("obsP", [128, BL], bf16, kind="ExternalInput")
    xT = nc.dram_tensor("xT", [D, BL], f32, kind="ExternalInput")
    xTb = nc.dram_tensor("xTb", [D, BL], bf16, kind="ExternalInput")
    ngT = nc.dram_tensor("ngT", [Z, BL], f32, kind="ExternalInput")
    pba = nc.dram_tensor("pba", [128, PBA_COLS], bf16, kind="ExternalInput")
    pbb = nc.dram_tensor("pbb", [128, PBB_COLS], bf16, kind="ExternalInput")
    pr = nc.dram_tensor("pr", [128, PR_COLS], f32r, kind="ExternalInput")
    pf = nc.dram_tensor("pf", [128, PF_COLS], f32, kind="ExternalInput")

    yT = nc.dram_tensor("yT", [2, BL], f32, kind="ExternalOutput")
    ztT = nc.dram_tensor("ztT", [Z, BL], f32, kind="ExternalOutput")
    kldg = nc.dram_tensor("kldg", [1, BL], f32, kind="ExternalOutput")
    klduT = nc.dram_tensor("klduT", [Z, BL], f32, kind="ExternalOutput")

    def rf(ap):
        return ap.bitcast(f32)

    with tile.TileContext(nc) as tc:
        with (
            tc.tile_pool(name="singles", bufs=1) as singles,
            tc.tile_pool(name="nz", bufs=2) as nzp,
            tc.tile_pool(name="h1", bufs=3) as h1p,
            tc.tile_pool(name="h2", bufs=3) as h2p,
            tc.tile_pool(name="wio", bufs=2) as wio,
            tc.tile_pool(name="wtmp", bufs=2) as wtmp,
            tc.tile_pool(name="psm1", bufs=3, space="PSUM") as psm1,
            tc.tile_pool(name="psm2", bufs=2, space="PSUM") as psm2,
            tc.tile_pool(name="psout", bufs=2, space="PSUM") as psout,
            tc.tile_pool(name="pssm", bufs=1, space="PSUM") as pssm,
        ):
            # ---- parameter blobs on the scalar HWDGE queue, by first use ----
            pba_s = singles.tile([128, PBA_COLS], bf16, tag="pba")
            nc.scalar.dma_start(out=pba_s, in_=pba[:, :])
            pf_s = singles.tile([128, PF_COLS], f32, tag="pf")
            nc.scalar.dma_start(out=pf_s, in_=pf[:, :])
            pbb_s = singles.tile([128, PBB_COLS], bf16, tag="pbb")
            nc.scalar.dma_start(out=pbb_s, in_=pbb[:, :])
            xTb_s = singles.tile([D, BL], bf16, tag="xTb")
            nc.scalar.dma_start(out=xTb_s, in_=xTb[:, :])
            pr_s = singles.tile([128, PR_COLS], f32r, tag="pr")
            nc.scalar.dma_start(out=pr_s, in_=pr[:, :])
            xT_s = singles.tile([D, BL], f32, tag="xT")
            nc.scalar.dma_start(out=xT_s, in_=xT[:, :])
            ngT_s = singles.tile([Z, BL], f32, tag="ngT")
            nc.scalar.dma_start(out=ngT_s, in_=ngT[:, :])

            wpair_v = [pba_s[:, q * PW:(q + 1) * PW] for q in range(32)]
            o = 0
            wobs_v = [pbb_s[:, o + g * 128: o + (g + 1) * 128] for g in range(8)]
            o += 8 * 128
            w2_v = [pbb_s[:, o + g * 128: o + (g + 1) * 128] for g in range(8)]
            o += 8 * 128
            wx_v = [pbb_s[0:D, o + g * 128: o + (g + 1) * 128] for g in range(8)]

            o = 0
            w3_v = [pr_s[:, o + g * 128: o + (g + 1) * 128] for g in range(8)]
            o += 8 * 128
            kw_s = pr_s[0:D, o:o + Z]; o += Z
            kg_s = pr_s[0:Z, o:o + 1]; o += 1
            wc1_s = pr_s[0:Z, o:o + HC1]; o += HC1
            wc2_s = pr_s[0:HC1, o:o + HC2]; o += HC2
            wc3_s = pr_s[0:HC2, o:o + 2]

            o = 0
            b1_v = [pf_s[:, o + g: o + g + 1] for g in range(8)]; o += 8
            b2_v = [pf_s[:, o + g: o + g + 1] for g in range(8)]; o += 8
            b3u_s = pf_s[0:Z, o:o + 1]; o += 1
            b3w_s = pf_s[0:Z, o:o + 1]; o += 1
            kbu_s = pf_s[0:Z, o:o + 1]; o += 1
            mg_s = pf_s[0:Z, o:o + 1]; o += 1
            sdg_s = pf_s[0:Z, o:o + 1]; o += 1
            kgb_s = pf_s[0:1, o:o + 1]; o += 1
            bc1_s = pf_s[0:HC1, o:o + 1]; o += 1
            bc2_s = pf_s[0:HC2, o:o + 1]; o += 1
            bc3_s = pf_s[0:2, o:o + 1]

            xsq_s = singles.tile([D, BL], f32r, tag="xsq")
            nc.vector.tensor_mul(xsq_s, xT_s, xT_s)

            for c in range(NCH):
                boff = CHB[c]
                CHc = CHS[c]
                # ---- noise DMA: 32KB contiguous runs per partition ----
                nsub = 8 if c == 0 else 4
                zw = 32 // nsub
                co = 32 * boff
                subs = []
                for kk in range(nsub):
                    nzk = nzp.tile([128, zw, CHc], bf16,
                                   tag=f"nz{'A' if c == 0 else 'B'}{kk}",
                                   bufs=1 if c == 0 else None)
                    nc.sync.dma_start(
                        out=nzk,
                        in_=nzP[:, co + kk * zw * CHc:co + (kk + 1) * zw * CHc])
                    subs.append(nzk)
                nzs = [(subs[q // zw], q % zw) for q in range(32)]
                noT = wio.tile([128, CHc], bf16, tag="noT")
                obs_ap = bass.AP(
                    tensor=obsP[:, :].tensor, offset=boff,
                    ap=[[BL, 128], [1, CHc]],
                )
                nc.sync.dma_start(out=noT, in_=obs_ap)

                latT_c = xT_s[0:Z, boff:boff + CHc]

                out_ps = psout.tile([128, CHc], f32, tag="outps")
                h1s, h2s = {}, {}

                def emit_m1(g):
                    m1 = psm1.tile([128, CHc], f32, tag="m1")
                    for j in range(4):
                        q = 4 * g + j
                        nc.tensor.matmul(
                            m1, wpair_v[q], nzs[q][0][:, nzs[q][1], :],
                            start=(j == 0), stop=False, skip_group_check=True,
                        )
                    nc.tensor.matmul(
                        m1, wx_v[g], xTb_s[:, boff:boff + CHc],
                        start=False, stop=False, skip_group_check=True,
                    )
                    nc.tensor.matmul(
                        m1, wobs_v[g], noT,
                        start=False, stop=True, skip_group_check=True,
                    )
                    h1 = h1p.tile([128, CHc], bf16, tag="h1")
                    if g % 2 == 0:
                        nc.scalar.activation(h1, m1, AF.Relu,
                                             bias=b1_v[g], scale=1.0)
                    else:
                        nc.vector.tensor_scalar(
                            out=h1, in0=m1, scalar1=b1_v[g],
                            scalar2=0.0, op0=OP.add, op1=OP.max)
                    h1s[g] = h1

                def emit_m2(g):
                    m2 = psm2.tile([128, CHc], f32, tag="m2")
                    nc.tensor.matmul(m2, w2_v[g], h1s[g], start=True, stop=True)
                    h2 = h2p.tile([128, CHc], f32r, tag="h2")
                    if g % 2 == 1:
                        nc.scalar.activation(h2, m2, AF.Relu,
                                             bias=b2_v[g], scale=1.0)
                    else:
                        nc.vector.tensor_scalar(
                            out=h2, in0=m2, scalar1=b2_v[g],
                            scalar2=0.0, op0=OP.add, op1=OP.max)
                    h2s[g] = h2

                def emit_m3(g):
                    nc.tensor.matmul(out_ps, w3_v[g], h2s[g],
                                     start=(g == 0), stop=(g == 7),
                                     skip_group_check=True)

                for g in range(8):
                    emit_m1(g)
                    emit_m2(g)
                    emit_m3(g)

                # ---- gated update; out_ps partitions: [0:64]=u, [64:128]=w
                w_t = wtmp.tile([Z, CHc], f32, tag="wt")
                nc.scalar.activation(w_t, out_ps[Z:128, :], AF.Sigmoid,
                                     bias=b3w_s, scale=1.0)
                t1 = wtmp.tile([Z, CHc], f32, tag="t1")
                nc.vector.scalar_tensor_tensor(
                    out=t1, in0=out_ps[0:Z, :], scalar=b3u_s, in1=latT_c,
                    op0=OP.add, op1=OP.subtract)
                t2 = wtmp.tile([Z, CHc], f32, tag="t2")
                nc.vector.tensor_mul(t2, t1, w_t)
                nl = wtmp.tile([Z, CHc], f32, tag="nl")
                nc.vector.tensor_add(nl, t2, latT_c)

                # ---- z_tilde = sd_g*noise_g + mult_g*new_lat
                t3 = wtmp.tile([Z, CHc], f32, tag="t3")
                nc.vector.tensor_scalar(
                    out=t3, in0=ngT_s[:, boff:boff + CHc], scalar1=sdg_s,
                    scalar2=None, op0=OP.mult)
                zt = wio.tile([Z, CHc], f32, tag="zt")
                nc.vector.scalar_tensor_tensor(
                    out=zt, in0=nl, scalar=mg_s, in1=t3,
                    op0=OP.mult, op1=OP.add)
                nc.scalar.dma_start(out=ztT[:, boff:boff + CHc], in_=zt)
                ztr = wtmp.tile([Z, CHc], f32r, tag="ztr")
                nc.vector.tensor_copy(ztr, zt)

                # ---- kld_u = 0.5*sum_d x^2 mult^2 - 0.5*C[z]
                ku_ps = pssm.tile([Z, CHc], f32, tag="sm")
                nc.tensor.matmul(ku_ps, kw_s, xsq_s[:, boff:boff + CHc],
                                 start=True, stop=True)
                ku = wio.tile([Z, CHc], f32, tag="ku")
                nc.vector.tensor_scalar(out=ku, in0=ku_ps, scalar1=kbu_s,
                                        scalar2=None, op0=OP.add)
                nc.scalar.dma_start(out=klduT[:, boff:boff + CHc], in_=ku)


                # ---- kld_g = 0.5*sum_z mult_g^2 nl^2 - 0.5*Cg
                nlsq = wtmp.tile([Z, CHc], f32r, tag="nlsq")
                nc.vector.tensor_mul(nlsq, nl, nl)
                kg_ps = pssm.tile([1, CHc], f32, tag="sm")
                nc.tensor.matmul(kg_ps, kg_s, nlsq, start=True, stop=True)
                kgo = wio.tile([1, CHc], f32, tag="kgo")
                nc.vector.tensor_scalar(out=kgo, in0=kg_ps, scalar1=kgb_s,
                                        scalar2=None, op0=OP.add)
                nc.scalar.dma_start(out=kldg[0:1, boff:boff + CHc], in_=kgo)

                # ---- choice MLP on z_tilde
                c1_ps = pssm.tile([HC1, CHc], f32, tag="sm")
                nc.tensor.matmul(c1_ps, wc1_s, ztr, start=True, stop=True)
                c1s = wtmp.tile([HC1, CHc], f32r, tag="c1s")
                nc.scalar.activation(c1s, c1_ps, AF.Relu, bias=bc1_s, scale=1.0)
                c2_ps = pssm.tile([HC2, CHc], f32, tag="sm")
                nc.tensor.matmul(c2_ps, wc2_s, c1s, start=True, stop=True)
                c2s = wtmp.tile([HC2, CHc], f32r, tag="c2s")
                nc.scalar.activation(c2s, c2_ps, AF.Relu, bias=bc2_s, scale=1.0)
                y_ps = pssm.tile([2, CHc], f32, tag="sm")
                nc.tensor.matmul(y_ps, wc3_s, c2s, start=True, stop=True)
                ys = wio.tile([2, CHc], f32, tag="ys")
                nc.scalar.activation(ys, y_ps, AF.Identity, bias=bc3_s, scale=1.0)
                nc.scalar.dma_start(out=yT[:, boff:boff + CHc], in_=ys)

    nc.compile()
    return nc


def _prep_params(inp):
    """Host-side folding of the tiny (<1MB) parameters into packed blobs."""
    f = np.float32
    logvar_u = np.asarray(inp["logvar_u"], f)       # [D, Z]
    mult_u = np.asarray(inp["mult_u"], f)           # [D, Z]
    W1 = np.asarray(inp["W1"], f)                   # [Z, D, H1]
    b1 = np.asarray(inp["b1"], f)
    W2 = np.asarray(inp["W2"], f)
    b2 = np.asarray(inp["b2"], f)
    W3 = np.asarray(inp["W3"], f)
    b3 = np.asarray(inp["b3"], f)
    logvar_g = np.asarray(inp["logvar_g"], f)
    mult_g = np.asarray(inp["mult_g"], f)

    var_u = np.exp(logvar_u)
    sd_u = np.sqrt(var_u)
    W1n = sd_u.T[:, :, None] * W1                   # [Z, D, H1]
    W1x = mult_u.T[:, :, None] * W1

    wpair = np.zeros((128, 32, PW), f)
    for q in range(32):
        co = 32 * (q % 4)
        for d in range(Z):
            wpair[2 * d, q, co:co + 16] = W1n[q, d, :]
            wpair[2 * d + 1, q, co + 16:co + 32] = W1n[q + 32, d, :]

    wobs = np.zeros((128, 8, 128), f)
    wx = np.zeros((128, 8, 128), f)
    b1p = np.zeros((128, 8), f)
    w2p = np.zeros((128, 8, 128), f)
    b2p = np.zeros((128, 8), f)
    w3p = np.zeros((128, 8, 128), f)
    for g in range(8):
        for zl in range(8):
            z = _z_of(g, zl)
            s = 16 * zl
            for dd in range(2):
                wobs[dd * Z + z, g, s:s + 16] = W1n[z, Z + dd, :]
            wx[:D, g, s:s + 16] = W1x[z]
            b1p[s:s + 16, g] = b1[z]
            w2p[s:s + 16, g, s:s + 16] = W2[z]
            b2p[s:s + 16, g] = b2[z]
            for o in range(2):
                w3p[s:s + 16, g, z + Z * o] = W3[z, :, o]

    C = np.sum(1.0 + logvar_u - var_u, axis=0)
    var_g = np.exp(logvar_g)
    Cg = np.sum(1.0 + logvar_g - var_g)

    # ---- pack blobs ----
    pba = wpair.reshape(128, 32 * PW).astype(BF16)
    pbb = np.concatenate([
        wobs.reshape(128, 8 * 128),
        w2p.reshape(128, 8 * 128),
        wx.reshape(128, 8 * 128),
    ], axis=1).astype(BF16)

    pr = np.zeros((128, PR_COLS), f)
    o = 0
    pr[:, o:o + 8 * 128] = w3p.reshape(128, 8 * 128); o += 8 * 128
    pr[:D, o:o + Z] = 0.5 * mult_u * mult_u; o += Z
    pr[:Z, o] = 0.5 * mult_g * mult_g; o += 1
    pr[:Z, o:o + HC1] = np.asarray(inp["Wc1"], f); o += HC1
    pr[:HC1, o:o + HC2] = np.asarray(inp["Wc2"], f); o += HC2
    pr[:HC2, o:o + 2] = np.asarray(inp["Wc3"], f)

    pf = np.zeros((128, PF_COLS), f)
    o = 0
    pf[:, o:o + 8] = b1p; o += 8
    pf[:, o:o + 8] = b2p; o += 8
    pf[:Z, o] = b3[:, 0]; o += 1
    pf[:Z, o] = b3[:, 1]; o += 1
    pf[:Z, o] = -0.5 * C; o += 1
    pf[:Z, o] = mult_g; o += 1
    pf[:Z, o] = np.sqrt(var_g); o += 1
    pf[0, o] = -0.5 * Cg; o += 1
    pf[:HC1, o] = np.asarray(inp["bc1"], f); o += 1
    pf[:HC2, o] = np.asarray(inp["bc2"], f); o += 1
    pf[:2, o] = np.asarray(inp["bc3"], f)

    return {"pba": pba, "pbb": pbb, "pr": pr, "pf": pf}


def _layout_noise(noise_core):
    """[BL, 66, 64] -> nzP [128, 32*BL] bf16 (p = 2d + (z>=32),
    chunk-blocked, b contiguous) and obsP [128, BL] bf16."""
    blocks = []
    for c in range(NCH):
        blk = noise_core[CHB[c]:CHB[c + 1], :Z, :].reshape(CHS[c], Z, 2, 32)
        blocks.append(blk.transpose(1, 2, 3, 0).reshape(128, 32 * CHS[c]))
    nzP = np.ascontiguousarray(np.concatenate(blocks, axis=1).astype(BF16))
    ob = noise_core[:, Z:, :]                        # [BL, 2, 64]
    obsP = np.ascontiguousarray(
        ob.transpose(1, 2, 0).reshape(128, BL).astype(BF16))
    return nzP, obsP


def kernel(**inputs):
    from concourse.bass_utils import run_bass_kernel_spmd

    if "nc" not in _cache:
        _cache["nc"] = _build()
    nc = _cache["nc"]

    f = np.float32
    latents = np.asarray(inputs["latents"], f)
    obs = np.asarray(inputs["obs"], f)
    t_0 = int(np.asarray(inputs["t_0"]))
    if t_0:
        lat = np.broadcast_to(np.asarray(inputs["z_0"], f), latents.shape)
    else:
        lat = latents
    x = np.concatenate([lat, obs], axis=-1)          # [B, D]
    xT = np.ascontiguousarray(x.T)
    ngT = np.ascontiguousarray(np.asarray(inputs["noise_global"], f).T)
    noise = np.asarray(inputs["noise_update"], f)

    params = _prep_params(inputs)

    in_maps = []
    for c in range(NCORES):
        s = slice(c * BL, (c + 1) * BL)
        m = dict(params)
        m["nzP"], m["obsP"] = _layout_noise(noise[s])
        m["xT"] = np.ascontiguousarray(xT[:, s])
        m["xTb"] = m["xT"].astype(BF16)
        m["ngT"] = np.ascontiguousarray(ngT[:, s])
        in_maps.append(m)

    res = run_bass_kernel_spmd(nc, in_maps, core_ids=list(range(NCORES)))
    _cache["last_results"] = res
    _cache["last_in_maps"] = in_maps

    y = np.concatenate([r["yT"] for r in res.results], axis=1).T
    z_tilde = np.concatenate([r["ztT"] for r in res.results], axis=1).T
    kld_g = np.concatenate([r["kldg"][0] for r in res.results], axis=0)
    kld_u = np.concatenate([r["klduT"] for r in res.results], axis=1).T
    return (np.ascontiguousarray(y), np.ascontiguousarray(z_tilde),
            kld_g, np.ascontiguousarray(kld_u))


# revision 25
# speedup vs baseline: 1.0245x; 1.0067x over previous
"""DisRNN sampling kernel for 8x Trainium2 NeuronCores (Bass/Tile).

Strategy (pure data parallel over batch B=16384 -> 2048 per core):
  The dominant cost is streaming noise_update [B, 66, 64] (~277MB fp32).
  All scaling factors (sqrt(var_u), mult_u) are folded into the per-latent
  MLP weights on the host, so the noise tensor feeds the tensor engine
  directly with no elementwise pass:
    h1[b,z,:] = relu( sum_d noise[b,d,z]*W1n[z,d,:]      (grouped matmul)
                    + sum_d x[b,d]  *W1x[z,d,:] + b1 )   (dense matmul)
  z-latents are packed pairwise (z, z+32) into K=128 block-diagonal
  matmuls. During sharding the host lays the noise slice out
  partition-major and chunk-blocked ([128 = 2d+(z>=32)] partitions, 32KB
  contiguous runs) so the device DMA runs at full HBM bandwidth and the
  matmul moving operand is contiguous. MLP2/MLP3 are 8-wide
  block-diagonal matmuls; kld_u / kld_g reduce to tiny matmuls on
  x^2 / new_latents^2. Noise-path matmuls run bf16, the rest float32r;
  all accumulation is fp32 in PSUM.
"""

import sys

if "/opt/trn_rl_repo" not in sys.path:
    sys.path.insert(0, "/opt/trn_rl_repo")

import numpy as np
import ml_dtypes

B, Z, NOBS = 16384, 64, 2
D = Z + NOBS
H1, H2 = 16, 16
HC1, HC2 = 32, 32
NCORES = 8
BL = B // NCORES          # 2048 per core
CHS = [512, 512, 512, 512]   # per-chunk batch sizes (sum = BL)
CHB = [0]
for _c in CHS:
    CHB.append(CHB[-1] + _c)
assert CHB[-1] == BL
NCH = len(CHS)
CH = 512                  # max chunk (PSUM free dim)

BF16 = ml_dtypes.bfloat16

# packed parameter blob column layouts (cols; all blobs have 128 partitions)
#   pba (bf16): wpair [32*PW];  pbb (bf16): wobs [8*128] | w2 [8*128] | wx [8*128]
#   pr (f32r): w3 [8*128] | kw [64] | kg [1] | wc1 [32] | wc2 [32] | wc3 [2]
#   pf (f32):  b1p [8] | b2p [8] | b3u,b3w,kbu,mg,sdg,kgb [6] | bc1,bc2,bc3 [3]
PW = 128
PBA_COLS = 32 * PW
PBB_COLS = 8 * 128 + 8 * 128 + 8 * 128
PR_COLS = 8 * 128 + 64 + 1 + 32 + 32 + 2
PF_COLS = 8 + 8 + 6 + 3

_cache = {}


def _z_of(g, zl):
    return (4 * g + zl // 2) + 32 * (zl % 2)


def _build():
    import concourse.bass as bass
    import concourse.mybir as mybir
    import concourse.tile as tile
    from concourse import bacc

    f32 = mybir.dt.float32
    f32r = mybir.dt.float32r
    bf16 = mybir.dt.bfloat16
    AF = mybir.ActivationFunctionType
    OP = mybir.AluOpType

    nc = bacc.Bacc("TRN2", target_bir_lowering=False, debug=False)

    nzP = nc.dram_tensor("nzP", [128, 32 * BL], bf16, kind="ExternalInput")
    obsP = nc.dram_tensor("obsP", [128, BL], bf16, kind="ExternalInput")
    xT = nc.dram_tensor("xT", [D, BL], f32, kind="ExternalInput")
    xTb = nc.dram_tensor("xTb", [D, BL], bf16, kind="ExternalInput")
    ngT = nc.dram_tensor("ngT", [Z, BL], f32, kind="ExternalInput")
    pba = nc.dram_tensor("pba", [128, PBA_COLS], bf16, kind="ExternalInput")
    pbb = nc.dram_tensor("pbb", [128, PBB_COLS], bf16, kind="ExternalInput")
    pr = nc.dram_tensor("pr", [128, PR_COLS], f32r, kind="ExternalInput")
    pf = nc.dram_tensor("pf", [128, PF_COLS], f32, kind="ExternalInput")

    yT = nc.dram_tensor("yT", [2, BL], f32, kind="ExternalOutput")
    ztT = nc.dram_tensor("ztT", [Z, BL], f32, kind="ExternalOutput")
    kldg = nc.dram_tensor("kldg", [1, BL], f32, kind="ExternalOutput")
    klduT = nc.dram_tensor("klduT", [Z, BL], f32, kind="ExternalOutput")

    def rf(ap):
        return ap.bitcast(f32)

    with tile.TileContext(nc) as tc:
        with (
            tc.tile_pool(name="singles", bufs=1) as singles,
            tc.tile_pool(name="nz", bufs=2) as nzp,
            tc.tile_pool(name="h1", bufs=3) as h1p,
            tc.tile_pool(name="h2", bufs=3) as h2p,
            tc.tile_pool(name="wio", bufs=2) as wio,
            tc.tile_pool(name="wtmp", bufs=2) as wtmp,
            tc.tile_pool(name="psm1", bufs=3, space="PSUM") as psm1,
            tc.tile_pool(name="psm2", bufs=2, space="PSUM") as psm2,
            tc.tile_pool(name="psout", bufs=2, space="PSUM") as psout,
            tc.tile_pool(name="pssm", bufs=1, space="PSUM") as pssm,
        ):
            # ---- parameter blobs on the scalar HWDGE queue, by first use ----
            pba_s = singles.tile([128, PBA_COLS], bf16, tag="pba")
            nc.scalar.dma_start(out=pba_s, in_=pba[:, :])
            pf_s = singles.tile([128, PF_COLS], f32, tag="pf")
            nc.scalar.dma_start(out=pf_s, in_=pf[:, :])
            pbb_s = singles.tile([128, PBB_COLS], bf16, tag="pbb")
            nc.scalar.dma_start(out=pbb_s, in_=pbb[:, :])
            xTb_s = singles.tile([D, BL], bf16, tag="xTb")
            nc.scalar.dma_start(out=xTb_s, in_=xTb[:, :])
            pr_s = singles.tile([128, PR_COLS], f32r, tag="pr")
            nc.scalar.dma_start(out=pr_s, in_=pr[:, :])
            xT_s = singles.tile([D, BL], f32, tag="xT")
            nc.scalar.dma_start(out=xT_s, in_=xT[:, :])
            ngT_s = singles.tile([Z, BL], f32, tag="ngT")
            nc.scalar.dma_start(out=ngT_s, in_=ngT[:, :])

            wpair_v = [pba_s[:, q * PW:(q + 1) * PW] for q in range(32)]
            o = 0
            wobs_v = [pbb_s[:, o + g * 128: o + (g + 1) * 128] for g in range(8)]
            o += 8 * 128
            w2_v = [pbb_s[:, o + g * 128: o + (g + 1) * 128] for g in range(8)]
            o += 8 * 128
            wx_v = [pbb_s[0:D, o + g * 128: o + (g + 1) * 128] for g in range(8)]

            o = 0
            w3_v = [pr_s[:, o + g * 128: o + (g + 1) * 128] for g in range(8)]
            o += 8 * 128
            kw_s = pr_s[0:D, o:o + Z]; o += Z
            kg_s = pr_s[0:Z, o:o + 1]; o += 1
            wc1_s = pr_s[0:Z, o:o + HC1]; o += HC1
            wc2_s = pr_s[0:HC1, o:o + HC2]; o += HC2
            wc3_s = pr_s[0:HC2, o:o + 2]

            o = 0
            b1_v = [pf_s[:, o + g: o + g + 1] for g in range(8)]; o += 8
            b2_v = [pf_s[:, o + g: o + g + 1] for g in range(8)]; o += 8
            b3u_s = pf_s[0:Z, o:o + 1]; o += 1
            b3w_s = pf_s[0:Z, o:o + 1]; o += 1
            kbu_s = pf_s[0:Z, o:o + 1]; o += 1
            mg_s = pf_s[0:Z, o:o + 1]; o += 1
            sdg_s = pf_s[0:Z, o:o + 1]; o += 1
            kgb_s = pf_s[0:1, o:o + 1]; o += 1
            bc1_s = pf_s[0:HC1, o:o + 1]; o += 1
            bc2_s = pf_s[0:HC2, o:o + 1]; o += 1
            bc3_s = pf_s[0:2, o:o + 1]

            xsq_s = singles.tile([D, BL], f32r, tag="xsq")
            nc.vector.tensor_mul(xsq_s, xT_s, xT_s)

            for c in range(NCH):
                boff = CHB[c]
                CHc = CHS[c]
                # ---- noise DMA: 32KB contiguous runs per partition ----
                nsub = 8 if c == 0 else 4
                zw = 32 // nsub
                co = 32 * boff
                subs = []
                for kk in range(nsub):
                    nzk = nzp.tile([128, zw, CHc], bf16,
                                   tag=f"nz{'A' if c == 0 else 'B'}{kk}",
                                   bufs=1 if c == 0 else None)
                    nc.sync.dma_start(
                        out=nzk,
                        in_=nzP[:, co + kk * zw * CHc:co + (kk + 1) * zw * CHc])
                    subs.append(nzk)
                nzs = [(subs[q // zw], q % zw) for q in range(32)]
                noT = wio.tile([128, CHc], bf16, tag="noT")
                obs_ap = bass.AP(
                    tensor=obsP[:, :].tensor, offset=boff,
                    ap=[[BL, 128], [1, CHc]],
                )
                nc.sync.dma_start(out=noT, in_=obs_ap)

                latT_c = xT_s[0:Z, boff:boff + CHc]

                out_ps = psout.tile([128, CHc], f32, tag="outps")
                h1s, h2s = {}, {}

                def emit_m1(g):
                    m1 = psm1.tile([128, CHc], f32, tag="m1")
                    for j in range(4):
                        q = 4 * g + j
                        nc.tensor.matmul(
                            m1, wpair_v[q], nzs[q][0][:, nzs[q][1], :],
                            start=(j == 0), stop=False, skip_group_check=True,
                        )
                    nc.tensor.matmul(
                        m1, wx_v[g], xTb_s[:, boff:boff + CHc],
                        start=False, stop=False, skip_group_check=True,
                    )
                    nc.tensor.matmul(
                        m1, wobs_v[g], noT,
                        start=False, stop=True, skip_group_check=True,
                    )
                    h1 = h1p.tile([128, CHc], bf16, tag="h1")
                    if g % 2 == 0:
                        nc.scalar.activation(h1, m1, AF.Relu,
                                             bias=b1_v[g], scale=1.0)
                    else:
                        nc.vector.tensor_scalar(
                            out=h1, in0=m1, scalar1=b1_v[g],
                            scalar2=0.0, op0=OP.add, op1=OP.max)
                    h1s[g] = h1

                def emit_m2(g):
                    m2 = psm2.tile([128, CHc], f32, tag="m2")
                    nc.tensor.matmul(m2, w2_v[g], h1s[g], start=True, stop=True)
                    h2 = h2p.tile([128, CHc], f32r, tag="h2")
                    if g % 2 == 1:
                        nc.scalar.activation(h2, m2, AF.Relu,
                                             bias=b2_v[g], scale=1.0)
                    else:
                        nc.vector.tensor_scalar(
                            out=h2, in0=m2, scalar1=b2_v[g],
                            scalar2=0.0, op0=OP.add, op1=OP.max)
                    h2s[g] = h2

                def emit_m3(g):
                    nc.tensor.matmul(out_ps, w3_v[g], h2s[g],
                                     start=(g == 0), stop=(g == 7),
                                     skip_group_check=True)

                for g in range(8):
                    emit_m1(g)
                    emit_m2(g)
                    emit_m3(g)

                # ---- gated update; out_ps partitions: [0:64]=u, [64:128]=w
                w_t = wtmp.tile([Z, CHc], f32, tag="wt")
                nc.scalar.activation(w_t, out_ps[Z:128, :], AF.Sigmoid,
                                     bias=b3w_s, scale=1.0)
                t1 = wtmp.tile([Z, CHc], f32, tag="t1")
                nc.vector.scalar_tensor_tensor(
                    out=t1, in0=out_ps[0:Z, :], scalar=b3u_s, in1=latT_c,
                    op0=OP.add, op1=OP.subtract)
                t2 = wtmp.tile([Z, CHc], f32, tag="t2")
                nc.vector.tensor_mul(t2, t1, w_t)
                nl = wtmp.tile([Z, CHc], f32, tag="nl")
                nc.vector.tensor_add(nl, t2, latT_c)

                # ---- z_tilde = sd_g*noise_g + mult_g*new_lat
                t3 = wtmp.tile([Z, CHc], f32, tag="t3")
                nc.vector.tensor_scalar(
                    out=t3, in0=ngT_s[:, boff:boff + CHc], scalar1=sdg_s,
                    scalar2=None, op0=OP.mult)
                zt = wio.tile([Z, CHc], f32, tag="zt")
                nc.vector.scalar_tensor_tensor(
                    out=zt, in0=nl, scalar=mg_s, in1=t3,
                    op0=OP.mult, op1=OP.add)
                nc.scalar.dma_start(out=ztT[:, boff:boff + CHc], in_=zt)
                ztr = wtmp.tile([Z, CHc], f32r, tag="ztr")
                nc.vector.tensor_copy(ztr, zt)

                # ---- kld_u = 0.5*sum_d x^2 mult^2 - 0.5*C[z]
                ku_ps = pssm.tile([Z, CHc], f32, tag="sm")
                nc.tensor.matmul(ku_ps, kw_s, xsq_s[:, boff:boff + CHc],
                                 start=True, stop=True)
                ku = wio.tile([Z, CHc], f32, tag="ku")
                nc.vector.tensor_scalar(out=ku, in0=ku_ps, scalar1=kbu_s,
                                        scalar2=None, op0=OP.add)
                nc.scalar.dma_start(out=klduT[:, boff:boff + CHc], in_=ku)


                # ---- kld_g = 0.5*sum_z mult_g^2 nl^2 - 0.5*Cg
                nlsq = wtmp.tile([Z, CHc], f32r, tag="nlsq")
                nc.vector.tensor_mul(nlsq, nl, nl)
                kg_ps = pssm.tile([1, CHc], f32, tag="sm")
                nc.tensor.matmul(kg_ps, kg_s, nlsq, start=True, stop=True)
                kgo = wio.tile([1, CHc], f32, tag="kgo")
                nc.vector.tensor_scalar(out=kgo, in0=kg_ps, scalar1=kgb_s,
                                        scalar2=None, op0=OP.add)
                nc.scalar.dma_start(out=kldg[0:1, boff:boff + CHc], in_=kgo)

                # ---- choice MLP on z_tilde
                c1_ps = pssm.tile([HC1, CHc], f32, tag="sm")
                nc.tensor.matmul(c1_ps, wc1_s, ztr, start=True, stop=True)
                c1s = wtmp.tile([HC1, CHc], f32r, tag="c1s")
                nc.scalar.activation(c1s, c1_ps, AF.Relu, bias=bc1_s, scale=1.0)
                c2_ps = pssm.tile([HC2, CHc], f32, tag="sm")
                nc.tensor.matmul(c2_ps, wc2_s, c1s, start=True, stop=True)
                c2s = wtmp.tile([HC2, CHc], f32r, tag="c2s")
                nc.scalar.activation(c2s, c2_ps, AF.Relu, bias=bc2_s, scale=1.0)
                y_ps = pssm.tile([2, CHc], f32, tag="sm")
                nc.tensor.matmul(y_ps, wc3_s, c2s, start=True, stop=True)
                ys = wio.tile([2, CHc], f32, tag="ys")
                nc.scalar.activation(ys, y_ps, AF.Identity, bias=bc3_s, scale=1.0)
                nc.scalar.dma_start(out=yT[:, boff:boff + CHc], in_=ys)

    nc.compile()
    return nc


def _prep_params(inp):
    """Host-side folding of the tiny (<1MB) parameters into packed blobs."""
    f = np.float32
    logvar_u = np.asarray(inp["logvar_u"], f)       # [D, Z]
    mult_u = np.asarray(inp["mult_u"], f)           # [D, Z]
    W1 = np.asarray(inp["W1"], f)                   # [Z, D, H1]
    b1 = np.asarray(inp["b1"], f)
    W2 = np.asarray(inp["W2"], f)
    b2 = np.asarray(inp["b2"], f)
    W3 = np.asarray(inp["W3"], f)
    b3 = np.asarray(inp["b3"], f)
    logvar_g = np.asarray(inp["logvar_g"], f)
    mult_g = np.asarray(inp["mult_g"], f)

    var_u = np.exp(logvar_u)
    sd_u = np.sqrt(var_u)
    W1n = sd_u.T[:, :, None] * W1                   # [Z, D, H1]
    W1x = mult_u.T[:, :, None] * W1

    wpair = np.zeros((128, 32, PW), f)
    for q in range(32):
        co = 32 * (q % 4)
        for d in range(Z):
            wpair[2 * d, q, co:co + 16] = W1n[q, d, :]
            wpair[2 * d + 1, q, co + 16:co + 32] = W1n[q + 32, d, :]

    wobs = np.zeros((128, 8, 128), f)
    wx = np.zeros((128, 8, 128), f)
    b1p = np.zeros((128, 8), f)
    w2p = np.zeros((128, 8, 128), f)
    b2p = np.zeros((128, 8), f)
    w3p = np.zeros((128, 8, 128), f)
    for g in range(8):
        for zl in range(8):
            z = _z_of(g, zl)
            s = 16 * zl
            for dd in range(2):
                wobs[dd * Z + z, g, s:s + 16] = W1n[z, Z + dd, :]
            wx[:D, g, s:s + 16] = W1x[z]
            b1p[s:s + 16, g] = b1[z]
            w2p[s:s + 16, g, s:s + 16] = W2[z]
            b2p[s:s + 16, g] = b2[z]
            for o in range(2):
                w3p[s:s + 16, g, z + Z * o] = W3[z, :, o]

    C = np.sum(1.0 + logvar_u - var_u, axis=0)
    var_g = np.exp(logvar_g)
    Cg = np.sum(1.0 + logvar_g - var_g)

    # ---- pack blobs ----
    pba = wpair.reshape(128, 32 * PW).astype(BF16)
    pbb = np.concatenate([
        wobs.reshape(128, 8 * 128),
        w2p.reshape(128, 8 * 128),
        wx.reshape(128, 8 * 128),
    ], axis=1).astype(BF16)

    pr = np.zeros((128, PR_COLS), f)
    o = 0
    pr[:, o:o + 8 * 128] = w3p.reshape(128, 8 * 128); o += 8 * 128
    pr[:D, o:o + Z] = 0.5 * mult_u * mult_u; o += Z
    pr[:Z, o] = 0.5 * mult_g * mult_g; o += 1
    pr[:Z, o:o + HC1] = np.asarray(inp["Wc1"], f); o += HC1
    pr[:HC1, o:o + HC2] = np.asarray(inp["Wc2"], f); o += HC2
    pr[:HC2, o:o + 2] = np.asarray(inp["Wc3"], f)

    pf = np.zeros((128, PF_COLS), f)
    o = 0
    pf[:, o:o + 8] = b1p; o += 8
    pf[:, o:o + 8] = b2p; o += 8
    pf[:Z, o] = b3[:, 0]; o += 1
    pf[:Z, o] = b3[:, 1]; o += 1
    pf[:Z, o] = -0.5 * C; o += 1
    pf[:Z, o] = mult_g; o += 1
    pf[:Z, o] = np.sqrt(var_g); o += 1
    pf[0, o] = -0.5 * Cg; o += 1
    pf[:HC1, o] = np.asarray(inp["bc1"], f); o += 1
    pf[:HC2, o] = np.asarray(inp["bc2"], f); o += 1
    pf[:2, o] = np.asarray(inp["bc3"], f)

    return {"pba": pba, "pbb": pbb, "pr": pr, "pf": pf}


def _layout_noise(noise_core):
    """[BL, 66, 64] -> nzP [128, 32*BL] bf16 (p = 2d + (z>=32),
    chunk-blocked, b contiguous) and obsP [128, BL] bf16."""
    blocks = []
    for c in range(NCH):
        blk = noise_core[CHB[c]:CHB[c + 1], :Z, :].reshape(CHS[c], Z, 2, 32)
        blocks.append(blk.transpose(1, 2, 3, 0).reshape(128, 32 * CHS[c]))
    nzP = np.ascontiguousarray(np.concatenate(blocks, axis=1).astype(BF16))
    ob = noise_core[:, Z:, :]                        # [BL, 2, 64]
    obsP = np.ascontiguousarray(
        ob.transpose(1, 2, 0).reshape(128, BL).astype(BF16))
    return nzP, obsP


def kernel(**inputs):
    from concourse.bass_utils import run_bass_kernel_spmd

    if "nc" not in _cache:
        _cache["nc"] = _build()
    nc = _cache["nc"]

    f = np.float32
    latents = np.asarray(inputs["latents"], f)
    obs = np.asarray(inputs["obs"], f)
    t_0 = int(np.asarray(inputs["t_0"]))
    if t_0:
        lat = np.broadcast_to(np.asarray(inputs["z_0"], f), latents.shape)
    else:
        lat = latents
    x = np.concatenate([lat, obs], axis=-1)          # [B, D]
    xT = np.ascontiguousarray(x.T)
    ngT = np.ascontiguousarray(np.asarray(inputs["noise_global"], f).T)
    noise = np.asarray(inputs["noise_update"], f)

    params = _prep_params(inputs)

    in_maps = []
    for c in range(NCORES):
        s = slice(c * BL, (c + 1) * BL)
        m = dict(params)
        m["nzP"], m["obsP"] = _layout_noise(noise[s])
        m["xT"] = np.ascontiguousarray(xT[:, s])
        m["xTb"] = m["xT"].astype(BF16)
        m["ngT"] = np.ascontiguousarray(ngT[:, s])
        in_maps.append(m)

    res = run_bass_kernel_spmd(nc, in_maps, core_ids=list(range(NCORES)))
    _cache["last_results"] = res
    _cache["last_in_maps"] = in_maps

    y = np.concatenate([r["yT"] for r in res.results], axis=1).T
    z_tilde = np.concatenate([r["ztT"] for r in res.results], axis=1).T
    kld_g = np.concatenate([r["kldg"][0] for r in res.results], axis=0)
    kld_u = np.concatenate([r["klduT"] for r in res.results], axis=1).T
    return (np.ascontiguousarray(y), np.ascontiguousarray(z_tilde),
            kld_g, np.ascontiguousarray(kld_u))


# revision 26
# speedup vs baseline: 1.0275x; 1.0030x over previous
"""DisRNN sampling kernel for 8x Trainium2 NeuronCores (Bass/Tile).

Strategy (pure data parallel over batch B=16384 -> 2048 per core):
  The dominant cost is streaming noise_update [B, 66, 64] (~277MB fp32).
  All scaling factors (sqrt(var_u), mult_u) are folded into the per-latent
  MLP weights on the host, so the noise tensor feeds the tensor engine
  directly with no elementwise pass:
    h1[b,z,:] = relu( sum_d noise[b,d,z]*W1n[z,d,:]      (grouped matmul)
                    + sum_d x[b,d]  *W1x[z,d,:] + b1 )   (dense matmul)
  z-latents are packed pairwise (z, z+32) into K=128 block-diagonal
  matmuls. During sharding the host lays the noise slice out
  partition-major and chunk-blocked ([128 = 2d+(z>=32)] partitions, 32KB
  contiguous runs) so the device DMA runs at full HBM bandwidth and the
  matmul moving operand is contiguous. MLP2/MLP3 are 8-wide
  block-diagonal matmuls; kld_u / kld_g reduce to tiny matmuls on
  x^2 / new_latents^2. Noise-path matmuls run bf16, the rest float32r;
  all accumulation is fp32 in PSUM.
"""

import sys

if "/opt/trn_rl_repo" not in sys.path:
    sys.path.insert(0, "/opt/trn_rl_repo")

import numpy as np
import ml_dtypes

B, Z, NOBS = 16384, 64, 2
D = Z + NOBS
H1, H2 = 16, 16
HC1, HC2 = 32, 32
NCORES = 8
BL = B // NCORES          # 2048 per core
CHS = [512, 512, 512, 512]   # per-chunk batch sizes (sum = BL)
CHB = [0]
for _c in CHS:
    CHB.append(CHB[-1] + _c)
assert CHB[-1] == BL
NCH = len(CHS)
CH = 512                  # max chunk (PSUM free dim)

BF16 = ml_dtypes.bfloat16

# packed parameter blob column layouts (cols; all blobs have 128 partitions)
#   pba (bf16): wpair [32*PW];  pbb (bf16): wobs|w2|wx|w3 [8*128 each]
#   pr (f32r): w3 [8*128] | kw [64] | kg [1] | wc1 [32] | wc2 [32] | wc3 [2]
#   pf (f32):  b1p [8] | b2p [8] | b3u,b3w,kbu,mg,sdg,kgb [6] | bc1,bc2,bc3 [3]
PW = 128
PBA_COLS = 32 * PW
PBB_COLS = 4 * 8 * 128
PR_COLS = 64 + 1 + 32 + 32 + 2
PF_COLS = 8 + 8 + 6 + 3

_cache = {}


def _z_of(g, zl):
    return (4 * g + zl // 2) + 32 * (zl % 2)


def _build():
    import concourse.bass as bass
    import concourse.mybir as mybir
    import concourse.tile as tile
    from concourse import bacc

    f32 = mybir.dt.float32
    f32r = mybir.dt.float32r
    bf16 = mybir.dt.bfloat16
    AF = mybir.ActivationFunctionType
    OP = mybir.AluOpType

    nc = bacc.Bacc("TRN2", target_bir_lowering=False, debug=False)

    nzP = nc.dram_tensor("nzP", [128, 32 * BL], bf16, kind="ExternalInput")
    obsP = nc.dram_tensor("obsP", [128, BL], bf16, kind="ExternalInput")
    xT = nc.dram_tensor("xT", [D, BL], f32, kind="ExternalInput")
    xTb = nc.dram_tensor("xTb", [D, BL], bf16, kind="ExternalInput")
    ngT = nc.dram_tensor("ngT", [Z, BL], f32, kind="ExternalInput")
    pba = nc.dram_tensor("pba", [128, PBA_COLS], bf16, kind="ExternalInput")
    pbb = nc.dram_tensor("pbb", [128, PBB_COLS], bf16, kind="ExternalInput")
    pr = nc.dram_tensor("pr", [128, PR_COLS], f32r, kind="ExternalInput")
    pf = nc.dram_tensor("pf", [128, PF_COLS], f32, kind="ExternalInput")

    yT = nc.dram_tensor("yT", [2, BL], f32, kind="ExternalOutput")
    ztT = nc.dram_tensor("ztT", [Z, BL], f32, kind="ExternalOutput")
    kldg = nc.dram_tensor("kldg", [1, BL], f32, kind="ExternalOutput")
    klduT = nc.dram_tensor("klduT", [Z, BL], f32, kind="ExternalOutput")

    def rf(ap):
        return ap.bitcast(f32)

    with tile.TileContext(nc) as tc:
        with (
            tc.tile_pool(name="singles", bufs=1) as singles,
            tc.tile_pool(name="nz", bufs=2) as nzp,
            tc.tile_pool(name="h1", bufs=3) as h1p,
            tc.tile_pool(name="h2", bufs=3) as h2p,
            tc.tile_pool(name="wio", bufs=2) as wio,
            tc.tile_pool(name="wtmp", bufs=2) as wtmp,
            tc.tile_pool(name="psm1", bufs=3, space="PSUM") as psm1,
            tc.tile_pool(name="psm2", bufs=2, space="PSUM") as psm2,
            tc.tile_pool(name="psout", bufs=2, space="PSUM") as psout,
            tc.tile_pool(name="pssm", bufs=1, space="PSUM") as pssm,
        ):
            # ---- parameter blobs on the scalar HWDGE queue, by first use ----
            pba_s = singles.tile([128, PBA_COLS], bf16, tag="pba")
            nc.scalar.dma_start(out=pba_s, in_=pba[:, :])
            pf_s = singles.tile([128, PF_COLS], f32, tag="pf")
            nc.scalar.dma_start(out=pf_s, in_=pf[:, :])
            pbb_s = singles.tile([128, PBB_COLS], bf16, tag="pbb")
            nc.scalar.dma_start(out=pbb_s, in_=pbb[:, :])
            xTb_s = singles.tile([D, BL], bf16, tag="xTb")
            nc.scalar.dma_start(out=xTb_s, in_=xTb[:, :])
            pr_s = singles.tile([128, PR_COLS], f32r, tag="pr")
            nc.scalar.dma_start(out=pr_s, in_=pr[:, :])
            xT_s = singles.tile([D, BL], f32, tag="xT")
            nc.scalar.dma_start(out=xT_s, in_=xT[:, :])
            ngT_s = singles.tile([Z, BL], f32, tag="ngT")
            nc.scalar.dma_start(out=ngT_s, in_=ngT[:, :])

            wpair_v = [pba_s[:, q * PW:(q + 1) * PW] for q in range(32)]
            o = 0
            wobs_v = [pbb_s[:, o + g * 128: o + (g + 1) * 128] for g in range(8)]
            o += 8 * 128
            w2_v = [pbb_s[:, o + g * 128: o + (g + 1) * 128] for g in range(8)]
            o += 8 * 128
            wx_v = [pbb_s[0:D, o + g * 128: o + (g + 1) * 128] for g in range(8)]
            o += 8 * 128
            w3_v = [pbb_s[:, o + g * 128: o + (g + 1) * 128] for g in range(8)]

            o = 0
            kw_s = pr_s[0:D, o:o + Z]; o += Z
            kg_s = pr_s[0:Z, o:o + 1]; o += 1
            wc1_s = pr_s[0:Z, o:o + HC1]; o += HC1
            wc2_s = pr_s[0:HC1, o:o + HC2]; o += HC2
            wc3_s = pr_s[0:HC2, o:o + 2]

            o = 0
            b1_v = [pf_s[:, o + g: o + g + 1] for g in range(8)]; o += 8
            b2_v = [pf_s[:, o + g: o + g + 1] for g in range(8)]; o += 8
            b3u_s = pf_s[0:Z, o:o + 1]; o += 1
            b3w_s = pf_s[0:Z, o:o + 1]; o += 1
            kbu_s = pf_s[0:Z, o:o + 1]; o += 1
            mg_s = pf_s[0:Z, o:o + 1]; o += 1
            sdg_s = pf_s[0:Z, o:o + 1]; o += 1
            kgb_s = pf_s[0:1, o:o + 1]; o += 1
            bc1_s = pf_s[0:HC1, o:o + 1]; o += 1
            bc2_s = pf_s[0:HC2, o:o + 1]; o += 1
            bc3_s = pf_s[0:2, o:o + 1]

            xsq_s = singles.tile([D, BL], f32r, tag="xsq")
            nc.vector.tensor_mul(xsq_s, xT_s, xT_s)

            for c in range(NCH):
                boff = CHB[c]
                CHc = CHS[c]
                # ---- noise DMA: 32KB contiguous runs per partition ----
                nsub = 8 if c == 0 else 4
                zw = 32 // nsub
                co = 32 * boff
                subs = []
                for kk in range(nsub):
                    nzk = nzp.tile([128, zw, CHc], bf16,
                                   tag=f"nz{'A' if c == 0 else 'B'}{kk}",
                                   bufs=1 if c == 0 else None)
                    nc.sync.dma_start(
                        out=nzk,
                        in_=nzP[:, co + kk * zw * CHc:co + (kk + 1) * zw * CHc])
                    subs.append(nzk)
                nzs = [(subs[q // zw], q % zw) for q in range(32)]
                noT = wio.tile([128, CHc], bf16, tag="noT")
                obs_ap = bass.AP(
                    tensor=obsP[:, :].tensor, offset=boff,
                    ap=[[BL, 128], [1, CHc]],
                )
                nc.sync.dma_start(out=noT, in_=obs_ap)

                latT_c = xT_s[0:Z, boff:boff + CHc]

                out_ps = psout.tile([128, CHc], f32, tag="outps")
                h1s, h2s = {}, {}

                def emit_m1(g):
                    m1 = psm1.tile([128, CHc], f32, tag="m1")
                    for j in range(4):
                        q = 4 * g + j
                        nc.tensor.matmul(
                            m1, wpair_v[q], nzs[q][0][:, nzs[q][1], :],
                            start=(j == 0), stop=False, skip_group_check=True,
                        )
                    nc.tensor.matmul(
                        m1, wx_v[g], xTb_s[:, boff:boff + CHc],
                        start=False, stop=False, skip_group_check=True,
                    )
                    nc.tensor.matmul(
                        m1, wobs_v[g], noT,
                        start=False, stop=True, skip_group_check=True,
                    )
                    h1 = h1p.tile([128, CHc], bf16, tag="h1")
                    if g % 2 == 0:
                        nc.scalar.activation(h1, m1, AF.Relu,
                                             bias=b1_v[g], scale=1.0)
                    else:
                        nc.vector.tensor_scalar(
                            out=h1, in0=m1, scalar1=b1_v[g],
                            scalar2=0.0, op0=OP.add, op1=OP.max)
                    h1s[g] = h1

                def emit_m2(g):
                    m2 = psm2.tile([128, CHc], f32, tag="m2")
                    nc.tensor.matmul(m2, w2_v[g], h1s[g], start=True, stop=True)
                    h2 = h2p.tile([128, CHc], bf16, tag="h2")
                    if g % 2 == 1:
                        nc.scalar.activation(h2, m2, AF.Relu,
                                             bias=b2_v[g], scale=1.0)
                    else:
                        nc.vector.tensor_scalar(
                            out=h2, in0=m2, scalar1=b2_v[g],
                            scalar2=0.0, op0=OP.add, op1=OP.max)
                    h2s[g] = h2

                def emit_m3(g):
                    nc.tensor.matmul(out_ps, w3_v[g], h2s[g],
                                     start=(g == 0), stop=(g == 7),
                                     skip_group_check=True)

                for g in range(8):
                    emit_m1(g)
                    emit_m2(g)
                    emit_m3(g)

                # ---- gated update; out_ps partitions: [0:64]=u, [64:128]=w
                w_t = wtmp.tile([Z, CHc], f32, tag="wt")
                nc.scalar.activation(w_t, out_ps[Z:128, :], AF.Sigmoid,
                                     bias=b3w_s, scale=1.0)
                t1 = wtmp.tile([Z, CHc], f32, tag="t1")
                nc.vector.scalar_tensor_tensor(
                    out=t1, in0=out_ps[0:Z, :], scalar=b3u_s, in1=latT_c,
                    op0=OP.add, op1=OP.subtract)
                t2 = wtmp.tile([Z, CHc], f32, tag="t2")
                nc.vector.tensor_mul(t2, t1, w_t)
                nl = wtmp.tile([Z, CHc], f32, tag="nl")
                nc.vector.tensor_add(nl, t2, latT_c)

                # ---- z_tilde = sd_g*noise_g + mult_g*new_lat
                t3 = wtmp.tile([Z, CHc], f32, tag="t3")
                nc.vector.tensor_scalar(
                    out=t3, in0=ngT_s[:, boff:boff + CHc], scalar1=sdg_s,
                    scalar2=None, op0=OP.mult)
                zt = wio.tile([Z, CHc], f32, tag="zt")
                nc.vector.scalar_tensor_tensor(
                    out=zt, in0=nl, scalar=mg_s, in1=t3,
                    op0=OP.mult, op1=OP.add)
                nc.scalar.dma_start(out=ztT[:, boff:boff + CHc], in_=zt)
                ztr = wtmp.tile([Z, CHc], f32r, tag="ztr")
                nc.vector.tensor_copy(ztr, zt)

                # ---- kld_u = 0.5*sum_d x^2 mult^2 - 0.5*C[z]
                ku_ps = pssm.tile([Z, CHc], f32, tag="sm")
                nc.tensor.matmul(ku_ps, kw_s, xsq_s[:, boff:boff + CHc],
                                 start=True, stop=True)
                ku = wio.tile([Z, CHc], f32, tag="ku")
                nc.vector.tensor_scalar(out=ku, in0=ku_ps, scalar1=kbu_s,
                                        scalar2=None, op0=OP.add)
                nc.scalar.dma_start(out=klduT[:, boff:boff + CHc], in_=ku)


                # ---- kld_g = 0.5*sum_z mult_g^2 nl^2 - 0.5*Cg
                nlsq = wtmp.tile([Z, CHc], f32r, tag="nlsq")
                nc.vector.tensor_mul(nlsq, nl, nl)
                kg_ps = pssm.tile([1, CHc], f32, tag="sm")
                nc.tensor.matmul(kg_ps, kg_s, nlsq, start=True, stop=True)
                kgo = wio.tile([1, CHc], f32, tag="kgo")
                nc.vector.tensor_scalar(out=kgo, in0=kg_ps, scalar1=kgb_s,
                                        scalar2=None, op0=OP.add)
                nc.scalar.dma_start(out=kldg[0:1, boff:boff + CHc], in_=kgo)

                # ---- choice MLP on z_tilde
                c1_ps = pssm.tile([HC1, CHc], f32, tag="sm")
                nc.tensor.matmul(c1_ps, wc1_s, ztr, start=True, stop=True)
                c1s = wtmp.tile([HC1, CHc], f32r, tag="c1s")
                nc.scalar.activation(c1s, c1_ps, AF.Relu, bias=bc1_s, scale=1.0)
                c2_ps = pssm.tile([HC2, CHc], f32, tag="sm")
                nc.tensor.matmul(c2_ps, wc2_s, c1s, start=True, stop=True)
                c2s = wtmp.tile([HC2, CHc], f32r, tag="c2s")
                nc.scalar.activation(c2s, c2_ps, AF.Relu, bias=bc2_s, scale=1.0)
                y_ps = pssm.tile([2, CHc], f32, tag="sm")
                nc.tensor.matmul(y_ps, wc3_s, c2s, start=True, stop=True)
                ys = wio.tile([2, CHc], f32, tag="ys")
                nc.scalar.activation(ys, y_ps, AF.Identity, bias=bc3_s, scale=1.0)
                nc.scalar.dma_start(out=yT[:, boff:boff + CHc], in_=ys)

    nc.compile()
    return nc


def _prep_params(inp):
    """Host-side folding of the tiny (<1MB) parameters into packed blobs."""
    f = np.float32
    logvar_u = np.asarray(inp["logvar_u"], f)       # [D, Z]
    mult_u = np.asarray(inp["mult_u"], f)           # [D, Z]
    W1 = np.asarray(inp["W1"], f)                   # [Z, D, H1]
    b1 = np.asarray(inp["b1"], f)
    W2 = np.asarray(inp["W2"], f)
    b2 = np.asarray(inp["b2"], f)
    W3 = np.asarray(inp["W3"], f)
    b3 = np.asarray(inp["b3"], f)
    logvar_g = np.asarray(inp["logvar_g"], f)
    mult_g = np.asarray(inp["mult_g"], f)

    var_u = np.exp(logvar_u)
    sd_u = np.sqrt(var_u)
    W1n = sd_u.T[:, :, None] * W1                   # [Z, D, H1]
    W1x = mult_u.T[:, :, None] * W1

    wpair = np.zeros((128, 32, PW), f)
    for q in range(32):
        co = 32 * (q % 4)
        for d in range(Z):
            wpair[2 * d, q, co:co + 16] = W1n[q, d, :]
            wpair[2 * d + 1, q, co + 16:co + 32] = W1n[q + 32, d, :]

    wobs = np.zeros((128, 8, 128), f)
    wx = np.zeros((128, 8, 128), f)
    b1p = np.zeros((128, 8), f)
    w2p = np.zeros((128, 8, 128), f)
    b2p = np.zeros((128, 8), f)
    w3p = np.zeros((128, 8, 128), f)
    for g in range(8):
        for zl in range(8):
            z = _z_of(g, zl)
            s = 16 * zl
            for dd in range(2):
                wobs[dd * Z + z, g, s:s + 16] = W1n[z, Z + dd, :]
            wx[:D, g, s:s + 16] = W1x[z]
            b1p[s:s + 16, g] = b1[z]
            w2p[s:s + 16, g, s:s + 16] = W2[z]
            b2p[s:s + 16, g] = b2[z]
            for o in range(2):
                w3p[s:s + 16, g, z + Z * o] = W3[z, :, o]

    C = np.sum(1.0 + logvar_u - var_u, axis=0)
    var_g = np.exp(logvar_g)
    Cg = np.sum(1.0 + logvar_g - var_g)

    # ---- pack blobs ----
    pba = wpair.reshape(128, 32 * PW).astype(BF16)
    pbb = np.concatenate([
        wobs.reshape(128, 8 * 128),
        w2p.reshape(128, 8 * 128),
        wx.reshape(128, 8 * 128),
        w3p.reshape(128, 8 * 128),
    ], axis=1).astype(BF16)

    pr = np.zeros((128, PR_COLS), f)
    o = 0
    pr[:D, o:o + Z] = 0.5 * mult_u * mult_u; o += Z
    pr[:Z, o] = 0.5 * mult_g * mult_g; o += 1
    pr[:Z, o:o + HC1] = np.asarray(inp["Wc1"], f); o += HC1
    pr[:HC1, o:o + HC2] = np.asarray(inp["Wc2"], f); o += HC2
    pr[:HC2, o:o + 2] = np.asarray(inp["Wc3"], f)

    pf = np.zeros((128, PF_COLS), f)
    o = 0
    pf[:, o:o + 8] = b1p; o += 8
    pf[:, o:o + 8] = b2p; o += 8
    pf[:Z, o] = b3[:, 0]; o += 1
    pf[:Z, o] = b3[:, 1]; o += 1
    pf[:Z, o] = -0.5 * C; o += 1
    pf[:Z, o] = mult_g; o += 1
    pf[:Z, o] = np.sqrt(var_g); o += 1
    pf[0, o] = -0.5 * Cg; o += 1
    pf[:HC1, o] = np.asarray(inp["bc1"], f); o += 1
    pf[:HC2, o] = np.asarray(inp["bc2"], f); o += 1
    pf[:2, o] = np.asarray(inp["bc3"], f)

    return {"pba": pba, "pbb": pbb, "pr": pr, "pf": pf}


def _layout_noise(noise_core):
    """[BL, 66, 64] -> nzP [128, 32*BL] bf16 (p = 2d + (z>=32),
    chunk-blocked, b contiguous) and obsP [128, BL] bf16."""
    blocks = []
    for c in range(NCH):
        blk = noise_core[CHB[c]:CHB[c + 1], :Z, :].reshape(CHS[c], Z, 2, 32)
        blocks.append(blk.transpose(1, 2, 3, 0).reshape(128, 32 * CHS[c]))
    nzP = np.ascontiguousarray(np.concatenate(blocks, axis=1).astype(BF16))
    ob = noise_core[:, Z:, :]                        # [BL, 2, 64]
    obsP = np.ascontiguousarray(
        ob.transpose(1, 2, 0).reshape(128, BL).astype(BF16))
    return nzP, obsP


def kernel(**inputs):
    from concourse.bass_utils import run_bass_kernel_spmd

    if "nc" not in _cache:
        _cache["nc"] = _build()
    nc = _cache["nc"]

    f = np.float32
    latents = np.asarray(inputs["latents"], f)
    obs = np.asarray(inputs["obs"], f)
    t_0 = int(np.asarray(inputs["t_0"]))
    if t_0:
        lat = np.broadcast_to(np.asarray(inputs["z_0"], f), latents.shape)
    else:
        lat = latents
    x = np.concatenate([lat, obs], axis=-1)          # [B, D]
    xT = np.ascontiguousarray(x.T)
    ngT = np.ascontiguousarray(np.asarray(inputs["noise_global"], f).T)
    noise = np.asarray(inputs["noise_update"], f)

    params = _prep_params(inputs)

    in_maps = []
    for c in range(NCORES):
        s = slice(c * BL, (c + 1) * BL)
        m = dict(params)
        m["nzP"], m["obsP"] = _layout_noise(noise[s])
        m["xT"] = np.ascontiguousarray(xT[:, s])
        m["xTb"] = m["xT"].astype(BF16)
        m["ngT"] = np.ascontiguousarray(ngT[:, s])
        in_maps.append(m)

    res = run_bass_kernel_spmd(nc, in_maps, core_ids=list(range(NCORES)))
    _cache["last_results"] = res
    _cache["last_in_maps"] = in_maps

    y = np.concatenate([r["yT"] for r in res.results], axis=1).T
    z_tilde = np.concatenate([r["ztT"] for r in res.results], axis=1).T
    kld_g = np.concatenate([r["kldg"][0] for r in res.results], axis=0)
    kld_u = np.concatenate([r["klduT"] for r in res.results], axis=1).T
    return (np.ascontiguousarray(y), np.ascontiguousarray(z_tilde),
            kld_g, np.ascontiguousarray(kld_u))
